# revision 11
# baseline (speedup 1.0000x reference)
"""TRN2 Bass kernel for batched compressed-sensing ISTA solver (nn_CS).

Reference semantics (per batch*channel signal of length N=2048, M=512
measurements at sorted unique indices `idxs`):
    b = SCALE * x[idxs]
    s_0 = 0
    repeat N_ITERS:                        # A = D[:, idxs], D = ortho DCT-II matrix
        r   = s @ A - b                    # A s  = idct(s)[idxs]
        s   = soft_threshold(s - r @ A.T, STEP*C_L1)
    out = (s @ D) / SCALE                  # idct(s) / SCALE

All 3072 solves are independent -> shard batch*channel over 8 NeuronCores
(384 rows each). Per core everything lives in SBUF; each iteration is two
matmul groups on the TensorEngine against the constant A (2048x512):
    p1[m]  = A[:,m-block]^T @ sT          (64 matmuls,  contraction N=2048)
    rT'    = bT - p1                      ( = -r^T )
    p2[n]  = A[n-block,:] @ rT'           (64 matmuls,  contraction M=512)
    u      = sT + p2                      ( = (s - r A^T)^T )
    sT     = u - clip(u, -t, t)           ( = soft_threshold(u, t) )
Matmuls run in float32r (full PE rate; fp32 runs at 1/4 rate) by default.

Everything is stored feature-major ([feature, batch] = partition x free);
host transposes x / output once (pure layout prep).
"""

import sys
import numpy as np

for _p in ("/opt/trn_rl_repo", "/root/.axon_site/_ro/trn_rl_repo"):
    if _p not in sys.path:
        sys.path.insert(0, _p)

import concourse.bass as bass  # noqa: E402
import concourse.bacc as bacc  # noqa: E402
import concourse.mybir as mybir  # noqa: E402
import concourse.tile as tile  # noqa: E402
from concourse.bass_utils import run_bass_kernel_spmd  # noqa: E402

# ---- problem constants (hardcoded per spec) --------------------------------
B, CH, N, M = 256, 12, 2048, 512
NCORES = 8
BC = B * CH                  # 3072 total solves
BL = BC // NCORES            # 384 solves per core
N_ITERS = 100
SCALE = 100.0
C_L1 = 0.1
STEP = 0.5
THR = STEP * C_L1            # 0.05 soft threshold
KCH = N // 128               # 16 chunks of the N axis
MCH = M // 128               # 4 chunks of the M axis

F32 = mybir.dt.float32
F32R = mybir.dt.float32r
ADD = mybir.AluOpType.add
MAXOP = mybir.AluOpType.max
MINOP = mybir.AluOpType.min
MULT = mybir.AluOpType.mult

_CACHE: dict = {}


def _dct_matrix(n: int) -> np.ndarray:
    """D with dct(v, norm='ortho') = D @ v; idct(v) = D.T @ v (row: s @ D)."""
    k = np.arange(n, dtype=np.float64)[:, None]
    j = np.arange(n, dtype=np.float64)[None, :]
    D = np.cos(np.pi * (2.0 * j + 1.0) * k / (2.0 * n))
    D[0, :] *= np.sqrt(1.0 / n)
    D[1:, :] *= np.sqrt(2.0 / n)
    return D


def _pack(mat: np.ndarray, nch: int) -> np.ndarray:
    """[nch*128, C] row-major -> [128, nch, C] partition-major SBUF layout."""
    r, c = mat.shape
    assert r == nch * 128
    return np.ascontiguousarray(
        mat.reshape(nch, 128, c).swapaxes(0, 1), dtype=np.float32
    )


def _build(n_iters: int, use_f32r: bool, final_f32r: bool):
    """Build + compile the per-core Bass program (identical on all cores)."""
    mmdt = F32R if use_f32r else F32
    fdt = F32R if final_f32r else F32

    nc = bacc.Bacc("TRN2", target_bir_lowering=False, debug=False,
                   num_devices=NCORES)

    x_d = nc.dram_tensor("xTpk", [128, KCH, BL], mmdt, kind="ExternalInput")
    a_d = nc.dram_tensor("Apk", [128, KCH, M], mmdt, kind="ExternalInput")
    at_d = nc.dram_tensor("ATpk", [128, MCH, N], mmdt, kind="ExternalInput")
    sel_d = nc.dram_tensor("SELpk", [128, KCH, M], mmdt, kind="ExternalInput")
    d_d = nc.dram_tensor("Dpk", [KCH, 128, KCH * 128], fdt,
                         kind="ExternalInput")
    eye_d = nc.dram_tensor("EYE", [128, 128], mmdt, kind="ExternalInput")
    o_d = nc.dram_tensor("outT", [N, BL], F32, kind="ExternalOutput")

    with tile.TileContext(nc) as tc:
        with (
            tc.tile_pool(name="const", bufs=1) as cpool,
            tc.tile_pool(name="bT", bufs=MCH) as bpool,
            tc.tile_pool(name="sT", bufs=KCH) as spool,
            tc.tile_pool(name="sh", bufs=KCH) as shpool,
            tc.tile_pool(name="rT", bufs=2 * MCH) as rpool,
            tc.tile_pool(name="u", bufs=5) as upool,
            tc.tile_pool(name="clip", bufs=5) as clpool,
            tc.tile_pool(name="a1", bufs=5) as apool,
            tc.tile_pool(name="o", bufs=2) as opool,
            tc.tile_pool(name="psA", bufs=MCH, space="PSUM") as psA,
            tc.tile_pool(name="psB", bufs=4, space="PSUM") as psB,
        ):
            a_t = cpool.tile([128, KCH, M], mmdt, tag="A")
            at_t = cpool.tile([128, MCH, N], mmdt, tag="AT")

            negthr = cpool.tile([128, 1], F32, tag="negthr", name="negthr")
            nc.gpsimd.memset(negthr[:], -THR)
            eye_t = cpool.tile([128, 128], mmdt, tag="eye", name="eye")
            nc.sync.dma_start(eye_t[:], eye_d[:])

            bT = [bpool.tile([128, BL], mmdt, tag="bT", name=f"bT{m}")
                  for m in range(MCH)]
            bTn = [bpool.tile([128, BL], mmdt, tag="bTn", name=f"bTn{m}")
                   for m in range(MCH)]

            # ---- init: bT[m] = (SCALE*Sel)^T @ xT (f32r; PE rounds) ----
            with (
                tc.tile_pool(name="initx", bufs=KCH) as xpool,
                tc.tile_pool(name="inits", bufs=3) as ipool,
            ):
                xks = []
                for k in range(KCH):
                    xk = xpool.tile([128, BL], mmdt, tag="xk", name=f"xk{k}")
                    nc.sync.dma_start(xk[:], x_d[:, k, :])
                    xks.append(xk)
                for m in range(MCH):
                    ps = psA.tile([128, BL], F32, tag="psA", name="psA_b")
                    for k in range(KCH):
                        selmk = ipool.tile([128, 128], mmdt, tag="selmk",
                                           name=f"selmk{m}_{k}")
                        nc.sync.dma_start(
                            selmk[:], sel_d[:, k, m * 128:(m + 1) * 128])
                        nc.tensor.matmul(ps[:], selmk[:], xks[k][:],
                                         start=(k == 0), stop=(k == KCH - 1))
                    nc.vector.tensor_copy(bT[m][:], ps[:])
                    nc.scalar.mul(bTn[m][:], ps[:], -1.0)
                # constant uploads, chunked for fine-grained deps
                for m in range(MCH):
                    nc.sync.dma_start(at_t[:, m, :], at_d[:, m, :])
                for k in range(KCH):
                    nc.sync.dma_start(a_t[:, k, :], a_d[:, k, :])

            def soft_update(ps2, sh_tile, s_mm_tile):
                # shadow = soft_threshold(shadow + ps2, THR)  [fp32, exact]
                # s_mm   = round_f32r(shadow)                 [PE operand]
                u = upool.tile([128, BL], F32, tag="u", name="u")
                if sh_tile.fresh:
                    nc.vector.tensor_copy(u[:], ps2[:])
                    sh_tile.fresh = False
                else:
                    nc.vector.tensor_add(u[:], sh_tile.t[:], ps2[:])
                # soft(u) = relu(u-t) + min(u+t, 0), split across ACT/DVE/Pool
                a1 = apool.tile([128, BL], F32, tag="a1", name="a1")
                nc.scalar.activation(a1[:], u[:],
                                     mybir.ActivationFunctionType.Relu,
                                     bias=negthr[:])
                m2 = clpool.tile([128, BL], F32, tag="clip", name="m2")
                nc.vector.tensor_scalar(m2[:], u[:], THR, 0.0, ADD, MINOP)
                nc.gpsimd.tensor_add(sh_tile.t[:], a1[:], m2[:])
                if s_mm_tile is not None:
                    # PE RNE-rounds raw fp32 bits on read (probe-verified),
                    # so a bit-copy into the f32r tile is equivalent to a
                    # rounding copy - and DMA engines are otherwise idle.
                    nc.sync.dma_start(s_mm_tile[:],
                                      sh_tile.t[:].bitcast(mmdt))

            class _Shadow:
                def __init__(self, t):
                    self.t = t
                    self.fresh = True

            shadow = [_Shadow(shpool.tile([128, BL], F32, tag="sh",
                                          name=f"sh{n}"))
                      for n in range(KCH)]

            # ---- iteration 1 (s0 = 0): u = A @ bT directly ----
            s_cur = [spool.tile([128, BL], mmdt, tag="sT", name=f"s0_{n}")
                     for n in range(KCH)]
            for n in range(KCH):
                ps2 = psB.tile([128, BL], F32, tag="psB", name="ps2")
                for m in range(MCH):
                    nc.tensor.matmul(
                        ps2[:],
                        at_t[:, m, n * 128:(n + 1) * 128],
                        bT[m][:],
                        start=(m == 0), stop=(m == MCH - 1))
                soft_update(ps2, shadow[n], s_cur[n])

            # ---- iterations 2..n_iters ----
            for it in range(1, n_iters):
                rT = [rpool.tile([128, BL], mmdt, tag="rT", name=f"rT{m}")
                      for m in range(MCH)]
                # k-major interleaved accumulation across 4 PSUM banks:
                # each s_mm chunk is consumed by 4 consecutive matmuls, so
                # the PE tracks the elementwise drain with slack.
                ps1s = [psA.tile([128, BL], F32, tag="psA", name=f"ps1_{m}")
                        for m in range(MCH)]
                for m in range(MCH):
                    nc.tensor.matmul(ps1s[m][:], eye_t[:], bTn[m][:],
                                     start=True, stop=False)
                for k in range(KCH):
                    for m in range(MCH):
                        nc.tensor.matmul(
                            ps1s[m][:],
                            a_t[:, k, m * 128:(m + 1) * 128],
                            s_cur[k][:],
                            start=False, stop=(k == KCH - 1))
                for m in range(MCH):
                    # psum = A^T s - bT; rT' = -(psum)
                    nc.scalar.mul(rT[m][:], ps1s[m][:], -1.0)
                last = (it == n_iters - 1)
                for n in range(KCH):
                    ps2 = psB.tile([128, BL], F32, tag="psB", name="ps2")
                    for m in range(MCH):
                        nc.tensor.matmul(
                            ps2[:],
                            at_t[:, m, n * 128:(n + 1) * 128],
                            rT[m][:],
                            start=(m == 0), stop=(m == MCH - 1))
                    soft_update(ps2, shadow[n],
                                None if (last and fdt != mmdt) else s_cur[n])

            # ---- final: outT[n-block] = D[:,n-block]^T @ sT / SCALE ----
            with tc.tile_pool(name="dstr", bufs=4) as dpool:
                if fdt != mmdt:
                    s_cur = [sh.t for sh in shadow]
                for n in range(KCH):
                    d_t = dpool.tile([128, KCH, 128], fdt, tag="D", name="dstr")
                    nc.sync.dma_start(d_t[:], d_d[n].rearrange(
                        "p (k c) -> p k c", k=KCH))
                    ps2 = psB.tile([128, BL], F32, tag="psB", name="ps2")
                    for k in range(KCH):
                        nc.tensor.matmul(
                            ps2[:],
                            d_t[:, k, :],
                            s_cur[k][:],
                            start=(k == 0), stop=(k == KCH - 1))
                    o = opool.tile([128, BL], F32, tag="o", name="o")
                    nc.vector.tensor_scalar(o[:], ps2[:], 1.0 / SCALE, None,
                                            MULT)
                    nc.sync.dma_start(o_d[n * 128:(n + 1) * 128, :], o[:])

    nc.compile()
    return nc


def _get_nc(n_iters=N_ITERS, use_f32r=True, final_f32r=True):
    key = (n_iters, use_f32r, final_f32r)
    if key not in _CACHE:
        _CACHE[key] = _build(*key)
    return _CACHE[key]


def _make_in_maps(x: np.ndarray, idxs: np.ndarray):
    idxs = np.asarray(idxs).astype(np.int64)
    D = _dct_matrix(N)
    A = D[:, idxs]                                   # [N, M]
    sel = np.zeros((N, M), dtype=np.float64)
    sel[idxs, np.arange(M)] = SCALE
    a_p = _pack(A.astype(np.float32), KCH)
    at_p = _pack(np.ascontiguousarray(A.T).astype(np.float32), MCH)
    sel_p = _pack(sel.astype(np.float32), KCH)
    Df = D.astype(np.float32)
    d_p = np.stack([
        np.ascontiguousarray(
            Df[:, n * 128:(n + 1) * 128].reshape(KCH, 128, 128)
            .swapaxes(0, 1).reshape(128, KCH * 128))
        for n in range(KCH)])

    xf = np.asarray(x, dtype=np.float32).reshape(BC, N)
    in_maps = []
    for c in range(NCORES):
        shard = xf[c * BL:(c + 1) * BL, :]           # [BL, N]
        xt = np.ascontiguousarray(shard.T)           # [N, BL]
        in_maps.append({
            "EYE": np.eye(128, dtype=np.float32),
            "xTpk": _pack(xt, KCH),
            "Apk": a_p,
            "ATpk": at_p,
            "SELpk": sel_p,
            "Dpk": d_p,
        })
    return in_maps


def _run(x, idxs, n_iters=N_ITERS, use_f32r=True, final_f32r=True,
         trace=False, **spmd_kwargs):
    nc = _get_nc(n_iters, use_f32r, final_f32r)
    in_maps = _make_in_maps(x, idxs)
    res = run_bass_kernel_spmd(nc, in_maps, list(range(NCORES)), trace=trace,
                               **spmd_kwargs)
    outs = []
    for c in range(NCORES):
        ot = res.results[c]["outT"]                  # [N, BL]
        outs.append(np.ascontiguousarray(ot.T))      # [BL, N]
    full = np.concatenate(outs, axis=0).reshape(B, CH, N).astype(np.float32)
    return full, res


def kernel(x, idxs):
    full, _ = _run(x, idxs)
    return (full,)


# revision 16
# speedup vs baseline: 1.0108x; 1.0108x over previous
"""TRN2 Bass kernel for batched compressed-sensing ISTA solver (nn_CS).

Reference semantics (per batch*channel signal of length N=2048, M=512
measurements at sorted unique indices `idxs`):
    b = SCALE * x[idxs]
    s_0 = 0
    repeat N_ITERS:                        # A = D[:, idxs], D = ortho DCT-II matrix
        r   = s @ A - b                    # A s  = idct(s)[idxs]
        s   = soft_threshold(s - r @ A.T, STEP*C_L1)
    out = (s @ D) / SCALE                  # idct(s) / SCALE

All 3072 solves are independent -> shard batch*channel over 8 NeuronCores
(384 rows each). Per core everything lives in SBUF; each iteration is two
matmul groups on the TensorEngine against the constant A (2048x512):
    p1[m]  = A[:,m-block]^T @ sT          (64 matmuls,  contraction N=2048)
    rT'    = bT - p1                      ( = -r^T )
    p2[n]  = A[n-block,:] @ rT'           (64 matmuls,  contraction M=512)
    u      = sT + p2                      ( = (s - r A^T)^T )
    sT     = u - clip(u, -t, t)           ( = soft_threshold(u, t) )
Matmuls run in float32r (full PE rate; fp32 runs at 1/4 rate) by default.

Everything is stored feature-major ([feature, batch] = partition x free);
host transposes x / output once (pure layout prep).
"""

import sys
import numpy as np

for _p in ("/opt/trn_rl_repo", "/root/.axon_site/_ro/trn_rl_repo"):
    if _p not in sys.path:
        sys.path.insert(0, _p)

import concourse.bass as bass  # noqa: E402
import concourse.bacc as bacc  # noqa: E402
import concourse.mybir as mybir  # noqa: E402
import concourse.tile as tile  # noqa: E402
from concourse.bass_utils import run_bass_kernel_spmd  # noqa: E402

# ---- problem constants (hardcoded per spec) --------------------------------
B, CH, N, M = 256, 12, 2048, 512
NCORES = 8
BC = B * CH                  # 3072 total solves
BL = BC // NCORES            # 384 solves per core
N_ITERS = 100
SCALE = 100.0
C_L1 = 0.1
STEP = 0.5
THR = STEP * C_L1            # 0.05 soft threshold
KCH = N // 128               # 16 chunks of the N axis
MCH = M // 128               # 4 chunks of the M axis

F32 = mybir.dt.float32
F32R = mybir.dt.float32r
ADD = mybir.AluOpType.add
MAXOP = mybir.AluOpType.max
MINOP = mybir.AluOpType.min
MULT = mybir.AluOpType.mult

_CACHE: dict = {}


def _dct_matrix(n: int) -> np.ndarray:
    """D with dct(v, norm='ortho') = D @ v; idct(v) = D.T @ v (row: s @ D)."""
    k = np.arange(n, dtype=np.float64)[:, None]
    j = np.arange(n, dtype=np.float64)[None, :]
    D = np.cos(np.pi * (2.0 * j + 1.0) * k / (2.0 * n))
    D[0, :] *= np.sqrt(1.0 / n)
    D[1:, :] *= np.sqrt(2.0 / n)
    return D


def _pack(mat: np.ndarray, nch: int) -> np.ndarray:
    """[nch*128, C] row-major -> [128, nch, C] partition-major SBUF layout."""
    r, c = mat.shape
    assert r == nch * 128
    return np.ascontiguousarray(
        mat.reshape(nch, 128, c).swapaxes(0, 1), dtype=np.float32
    )


def _build(n_iters: int, use_f32r: bool, final_f32r: bool):
    """Build + compile the per-core Bass program (identical on all cores)."""
    mmdt = F32R if use_f32r else F32
    fdt = F32R if final_f32r else F32

    nc = bacc.Bacc("TRN2", target_bir_lowering=False, debug=False,
                   num_devices=NCORES)

    x_d = nc.dram_tensor("xTpk", [128, KCH, BL], mmdt, kind="ExternalInput")
    a_d = nc.dram_tensor("Apk", [128, KCH, M], mmdt, kind="ExternalInput")
    at_d = nc.dram_tensor("ATpk", [128, MCH, N], mmdt, kind="ExternalInput")
    sel_d = nc.dram_tensor("SELpk", [MCH, 128, KCH * 128], mmdt,
                           kind="ExternalInput")
    d_d = nc.dram_tensor("Dpk", [KCH, 128, KCH * 128], fdt,
                         kind="ExternalInput")
    eye_d = nc.dram_tensor("EYE", [128, 128], mmdt, kind="ExternalInput")
    o_d = nc.dram_tensor("outT", [N, BL], F32, kind="ExternalOutput")

    with tile.TileContext(nc) as tc:
        with (
            tc.tile_pool(name="const", bufs=1) as cpool,
            tc.tile_pool(name="bT", bufs=MCH) as bpool,
            tc.tile_pool(name="sT", bufs=KCH) as spool,
            tc.tile_pool(name="sh", bufs=KCH) as shpool,
            tc.tile_pool(name="rT", bufs=2 * MCH) as rpool,
            tc.tile_pool(name="u", bufs=4) as upool,
            tc.tile_pool(name="clip", bufs=4) as clpool,
            tc.tile_pool(name="a1", bufs=4) as apool,
            tc.tile_pool(name="o", bufs=2) as opool,
            tc.tile_pool(name="psA", bufs=MCH, space="PSUM") as psA,
            tc.tile_pool(name="psB", bufs=4, space="PSUM") as psB,
        ):
            a_t = cpool.tile([128, KCH, M], mmdt, tag="A")
            at_t = cpool.tile([128, MCH, N], mmdt, tag="AT")

            negthr = cpool.tile([128, 1], F32, tag="negthr", name="negthr")
            nc.gpsimd.memset(negthr[:], -THR)
            eye_t = cpool.tile([128, 128], mmdt, tag="eye", name="eye")
            nc.sync.dma_start(eye_t[:], eye_d[:])

            bTn = [bpool.tile([128, BL], mmdt, tag="bTn", name=f"bTn{m}")
                   for m in range(MCH)]

            # ---- init: bTn[m] = -(SCALE*Sel)^T @ xT (f32r; PE rounds) ----
            with (
                tc.tile_pool(name="initx", bufs=1) as xpool,
                tc.tile_pool(name="inits", bufs=2) as ipool,
            ):
                xfull = xpool.tile([128, KCH, BL], mmdt, tag="xk",
                                   name="xfull")
                nc.sync.dma_start(xfull[:], x_d[:])
                for m in range(MCH):
                    selm = ipool.tile([128, KCH * 128], mmdt, tag="selm",
                                      name=f"selm{m}")
                    nc.sync.dma_start(selm[:], sel_d[m])
                    ps = psA.tile([128, BL], F32, tag="psA", name="psA_b")
                    for k in range(KCH):
                        nc.tensor.matmul(ps[:],
                                         selm[:, k * 128:(k + 1) * 128],
                                         xfull[:, k, :],
                                         start=(k == 0), stop=(k == KCH - 1))
                    nc.scalar.mul(bTn[m][:], ps[:], -1.0)
                # constant uploads, batched with per-slice deps
                for m in range(MCH):
                    nc.gpsimd.dma_start(at_t[:, m, :], at_d[:, m, :])
                for g in range(4):
                    nc.sync.dma_start(a_t[:, 4 * g:4 * g + 4, :],
                                      a_d[:, 4 * g:4 * g + 4, :])

            def soft_update(ps2, sh_tile, s_mm_tile):
                # shadow = soft_threshold(shadow + ps2, THR)  [fp32, exact]
                # s_mm   = round_f32r(shadow)                 [PE operand]
                u = upool.tile([128, BL], F32, tag="u", name="u")
                if sh_tile.fresh:
                    # iteration 1 accumulated A @ (-b): negate into u
                    nc.vector.tensor_scalar(u[:], ps2[:], -1.0, None, MULT)
                    sh_tile.fresh = False
                else:
                    nc.vector.tensor_add(u[:], sh_tile.t[:], ps2[:])
                # soft(u) = relu(u-t) + min(u+t, 0), split across ACT/DVE/Pool
                a1 = apool.tile([128, BL], F32, tag="a1", name="a1")
                nc.scalar.activation(a1[:], u[:],
                                     mybir.ActivationFunctionType.Relu,
                                     bias=negthr[:])
                m2 = clpool.tile([128, BL], F32, tag="clip", name="m2")
                nc.vector.tensor_scalar(m2[:], u[:], THR, 0.0, ADD, MINOP)
                nc.gpsimd.tensor_add(sh_tile.t[:], a1[:], m2[:])
                if s_mm_tile is not None:
                    # PE RNE-rounds raw fp32 bits on read (probe-verified),
                    # so a bit-copy into the f32r tile is equivalent to a
                    # rounding copy - and DMA engines are otherwise idle.
                    nc.sync.dma_start(s_mm_tile[:],
                                      sh_tile.t[:].bitcast(mmdt))

            class _Shadow:
                def __init__(self, t):
                    self.t = t
                    self.fresh = True

            shadow = [_Shadow(shpool.tile([128, BL], F32, tag="sh",
                                          name=f"sh{n}"))
                      for n in range(KCH)]

            # ---- iteration 1 (s0 = 0): u = A @ bT directly ----
            s_cur = [spool.tile([128, BL], mmdt, tag="sT", name=f"s0_{n}")
                     for n in range(KCH)]
            for n in range(KCH):
                ps2 = psB.tile([128, BL], F32, tag="psB", name="ps2")
                for m in range(MCH):
                    nc.tensor.matmul(
                        ps2[:],
                        at_t[:, m, n * 128:(n + 1) * 128],
                        bTn[m][:],
                        start=(m == 0), stop=(m == MCH - 1))
                soft_update(ps2, shadow[n], s_cur[n])

            # ---- iterations 2..n_iters ----
            for it in range(1, n_iters):
                rT = [rpool.tile([128, BL], mmdt, tag="rT", name=f"rT{m}")
                      for m in range(MCH)]
                # k-major interleaved accumulation across 4 PSUM banks:
                # each s_mm chunk is consumed by 4 consecutive matmuls, so
                # the PE tracks the elementwise drain with slack.
                ps1s = [psA.tile([128, BL], F32, tag="psA", name=f"ps1_{m}")
                        for m in range(MCH)]
                for m in range(MCH):
                    nc.tensor.matmul(ps1s[m][:], eye_t[:], bTn[m][:],
                                     start=True, stop=False)
                for k in range(KCH):
                    for m in range(MCH):
                        nc.tensor.matmul(
                            ps1s[m][:],
                            a_t[:, k, m * 128:(m + 1) * 128],
                            s_cur[k][:],
                            start=False, stop=(k == KCH - 1))
                for m in range(MCH):
                    # psum = A^T s - bT; rT' = -(psum)
                    nc.scalar.mul(rT[m][:], ps1s[m][:], -1.0)
                last = (it == n_iters - 1)
                for n in range(KCH):
                    ps2 = psB.tile([128, BL], F32, tag="psB", name="ps2")
                    for m in range(MCH):
                        nc.tensor.matmul(
                            ps2[:],
                            at_t[:, m, n * 128:(n + 1) * 128],
                            rT[m][:],
                            start=(m == 0), stop=(m == MCH - 1))
                    soft_update(ps2, shadow[n],
                                None if (last and fdt != mmdt) else s_cur[n])

            # ---- final: outT[n-block] = D[:,n-block]^T @ sT / SCALE ----
            with tc.tile_pool(name="dstr", bufs=4) as dpool:
                if fdt != mmdt:
                    s_cur = [sh.t for sh in shadow]
                for n in range(KCH):
                    d_t = dpool.tile([128, KCH, 128], fdt, tag="D", name="dstr")
                    nc.gpsimd.dma_start(d_t[:], d_d[n].rearrange(
                        "p (k c) -> p k c", k=KCH))
                    ps2 = psB.tile([128, BL], F32, tag="psB", name="ps2")
                    for k in range(KCH):
                        nc.tensor.matmul(
                            ps2[:],
                            d_t[:, k, :],
                            s_cur[k][:],
                            start=(k == 0), stop=(k == KCH - 1))
                    o = opool.tile([128, BL], F32, tag="o", name="o")
                    nc.vector.tensor_scalar(o[:], ps2[:], 1.0 / SCALE, None,
                                            MULT)
                    nc.sync.dma_start(o_d[n * 128:(n + 1) * 128, :], o[:])

    nc.compile()
    return nc


def _get_nc(n_iters=N_ITERS, use_f32r=True, final_f32r=True):
    key = (n_iters, use_f32r, final_f32r)
    if key not in _CACHE:
        _CACHE[key] = _build(*key)
    return _CACHE[key]


def _make_in_maps(x: np.ndarray, idxs: np.ndarray):
    idxs = np.asarray(idxs).astype(np.int64)
    D = _dct_matrix(N)
    A = D[:, idxs]                                   # [N, M]
    sel = np.zeros((N, M), dtype=np.float64)
    sel[idxs, np.arange(M)] = SCALE
    a_p = _pack(A.astype(np.float32), KCH)
    at_p = _pack(np.ascontiguousarray(A.T).astype(np.float32), MCH)
    self32 = sel.astype(np.float32)
    sel_p = np.stack([
        np.ascontiguousarray(
            self32[:, m * 128:(m + 1) * 128].reshape(KCH, 128, 128)
            .swapaxes(0, 1).reshape(128, KCH * 128))
        for m in range(MCH)])
    Df = D.astype(np.float32)
    d_p = np.stack([
        np.ascontiguousarray(
            Df[:, n * 128:(n + 1) * 128].reshape(KCH, 128, 128)
            .swapaxes(0, 1).reshape(128, KCH * 128))
        for n in range(KCH)])

    xf = np.asarray(x, dtype=np.float32).reshape(BC, N)
    in_maps = []
    for c in range(NCORES):
        shard = xf[c * BL:(c + 1) * BL, :]           # [BL, N]
        xt = np.ascontiguousarray(shard.T)           # [N, BL]
        in_maps.append({
            "EYE": np.eye(128, dtype=np.float32),
            "xTpk": _pack(xt, KCH),
            "Apk": a_p,
            "ATpk": at_p,
            "SELpk": sel_p,
            "Dpk": d_p,
        })
    return in_maps


def _run(x, idxs, n_iters=N_ITERS, use_f32r=True, final_f32r=True,
         trace=False, **spmd_kwargs):
    nc = _get_nc(n_iters, use_f32r, final_f32r)
    in_maps = _make_in_maps(x, idxs)
    res = run_bass_kernel_spmd(nc, in_maps, list(range(NCORES)), trace=trace,
                               **spmd_kwargs)
    outs = []
    for c in range(NCORES):
        ot = res.results[c]["outT"]                  # [N, BL]
        outs.append(np.ascontiguousarray(ot.T))      # [BL, N]
    full = np.concatenate(outs, axis=0).reshape(B, CH, N).astype(np.float32)
    return full, res


def kernel(x, idxs):
    full, _ = _run(x, idxs)
    return (full,)


# revision 17
# speedup vs baseline: 1.0112x; 1.0004x over previous
"""TRN2 Bass kernel for batched compressed-sensing ISTA solver (nn_CS).

Reference semantics (per batch*channel signal of length N=2048, M=512
measurements at sorted unique indices `idxs`):
    b = SCALE * x[idxs]
    s_0 = 0
    repeat N_ITERS:                        # A = D[:, idxs], D = ortho DCT-II matrix
        r   = s @ A - b                    # A s  = idct(s)[idxs]
        s   = soft_threshold(s - r @ A.T, STEP*C_L1)
    out = (s @ D) / SCALE                  # idct(s) / SCALE

All 3072 solves are independent -> shard batch*channel over 8 NeuronCores
(384 rows each). Per core everything lives in SBUF; each iteration is two
matmul groups on the TensorEngine against the constant A (2048x512):
    p1[m]  = A[:,m-block]^T @ sT          (64 matmuls,  contraction N=2048)
    rT'    = bT - p1                      ( = -r^T )
    p2[n]  = A[n-block,:] @ rT'           (64 matmuls,  contraction M=512)
    u      = sT + p2                      ( = (s - r A^T)^T )
    sT     = u - clip(u, -t, t)           ( = soft_threshold(u, t) )
Matmuls run in float32r (full PE rate; fp32 runs at 1/4 rate) by default.

Everything is stored feature-major ([feature, batch] = partition x free);
host transposes x / output once (pure layout prep).
"""

import sys
import numpy as np

for _p in ("/opt/trn_rl_repo", "/root/.axon_site/_ro/trn_rl_repo"):
    if _p not in sys.path:
        sys.path.insert(0, _p)

import concourse.bass as bass  # noqa: E402
import concourse.bacc as bacc  # noqa: E402
import concourse.mybir as mybir  # noqa: E402
import concourse.tile as tile  # noqa: E402
from concourse.bass_utils import run_bass_kernel_spmd  # noqa: E402

# ---- problem constants (hardcoded per spec) --------------------------------
B, CH, N, M = 256, 12, 2048, 512
NCORES = 8
BC = B * CH                  # 3072 total solves
BL = BC // NCORES            # 384 solves per core
N_ITERS = 100
SCALE = 100.0
C_L1 = 0.1
STEP = 0.5
THR = STEP * C_L1            # 0.05 soft threshold
KCH = N // 128               # 16 chunks of the N axis
MCH = M // 128               # 4 chunks of the M axis

F32 = mybir.dt.float32
F32R = mybir.dt.float32r
ADD = mybir.AluOpType.add
MAXOP = mybir.AluOpType.max
MINOP = mybir.AluOpType.min
MULT = mybir.AluOpType.mult

_CACHE: dict = {}


def _dct_matrix(n: int) -> np.ndarray:
    """D with dct(v, norm='ortho') = D @ v; idct(v) = D.T @ v (row: s @ D)."""
    k = np.arange(n, dtype=np.float64)[:, None]
    j = np.arange(n, dtype=np.float64)[None, :]
    D = np.cos(np.pi * (2.0 * j + 1.0) * k / (2.0 * n))
    D[0, :] *= np.sqrt(1.0 / n)
    D[1:, :] *= np.sqrt(2.0 / n)
    return D


def _pack(mat: np.ndarray, nch: int) -> np.ndarray:
    """[nch*128, C] row-major -> [128, nch, C] partition-major SBUF layout."""
    r, c = mat.shape
    assert r == nch * 128
    return np.ascontiguousarray(
        mat.reshape(nch, 128, c).swapaxes(0, 1), dtype=np.float32
    )


def _build(n_iters: int, use_f32r: bool, final_f32r: bool):
    """Build + compile the per-core Bass program (identical on all cores)."""
    mmdt = F32R if use_f32r else F32
    fdt = F32R if final_f32r else F32

    nc = bacc.Bacc("TRN2", target_bir_lowering=False, debug=False,
                   num_devices=NCORES)

    x_d = nc.dram_tensor("xTpk", [128, KCH, BL], mmdt, kind="ExternalInput")
    a_d = nc.dram_tensor("Apk", [128, KCH, M], mmdt, kind="ExternalInput")
    at_d = nc.dram_tensor("ATpk", [128, MCH, N], mmdt, kind="ExternalInput")
    sel_d = nc.dram_tensor("SELpk", [MCH, 128, KCH * 128], mmdt,
                           kind="ExternalInput")
    d_d = nc.dram_tensor("Dpk", [KCH, 128, KCH * 128], fdt,
                         kind="ExternalInput")
    o_d = nc.dram_tensor("outT", [N, BL], F32, kind="ExternalOutput")

    with tile.TileContext(nc) as tc:
        with (
            tc.tile_pool(name="const", bufs=1) as cpool,
            tc.tile_pool(name="bT", bufs=MCH) as bpool,
            tc.tile_pool(name="sT", bufs=KCH) as spool,
            tc.tile_pool(name="sh", bufs=KCH) as shpool,
            tc.tile_pool(name="rT", bufs=2 * MCH) as rpool,
            tc.tile_pool(name="u", bufs=4) as upool,
            tc.tile_pool(name="clip", bufs=4) as clpool,
            tc.tile_pool(name="a1", bufs=4) as apool,
            tc.tile_pool(name="o", bufs=2) as opool,
            tc.tile_pool(name="psA", bufs=MCH, space="PSUM") as psA,
            tc.tile_pool(name="psB", bufs=4, space="PSUM") as psB,
        ):
            a_t = cpool.tile([128, KCH, M], mmdt, tag="A")
            at_t = cpool.tile([128, MCH, N], mmdt, tag="AT")

            negthr = cpool.tile([128, 1], F32, tag="negthr", name="negthr")
            nc.gpsimd.memset(negthr[:], -THR)

            bT = [bpool.tile([128, BL], mmdt, tag="bT", name=f"bT{m}")
                  for m in range(MCH)]

            # ---- init: bT[m] = (SCALE*Sel)^T @ xT (f32r; PE rounds) ----
            with (
                tc.tile_pool(name="initx", bufs=1) as xpool,
                tc.tile_pool(name="inits", bufs=2) as ipool,
            ):
                xfull = xpool.tile([128, KCH, BL], mmdt, tag="xk",
                                   name="xfull")
                nc.sync.dma_start(xfull[:], x_d[:])
                for m in range(MCH):
                    selm = ipool.tile([128, KCH * 128], mmdt, tag="selm",
                                      name=f"selm{m}")
                    nc.sync.dma_start(selm[:], sel_d[m])
                    ps = psA.tile([128, BL], F32, tag="psA", name="psA_b")
                    for k in range(KCH):
                        nc.tensor.matmul(ps[:],
                                         selm[:, k * 128:(k + 1) * 128],
                                         xfull[:, k, :],
                                         start=(k == 0), stop=(k == KCH - 1))
                    nc.vector.tensor_copy(bT[m][:], ps[:])
                # constant uploads, batched with per-slice deps
                for m in range(MCH):
                    nc.gpsimd.dma_start(at_t[:, m, :], at_d[:, m, :])
                for g in range(4):
                    nc.sync.dma_start(a_t[:, 4 * g:4 * g + 4, :],
                                      a_d[:, 4 * g:4 * g + 4, :])

            def soft_update(ps2, sh_tile, s_mm_tile):
                # shadow = soft_threshold(shadow + ps2, THR)  [fp32, exact]
                # s_mm   = round_f32r(shadow)                 [PE operand]
                u = upool.tile([128, BL], F32, tag="u", name="u")
                if sh_tile.fresh:
                    nc.vector.tensor_copy(u[:], ps2[:])
                    sh_tile.fresh = False
                else:
                    nc.vector.tensor_add(u[:], sh_tile.t[:], ps2[:])
                # soft(u) = relu(u-t) + min(u+t, 0), split across ACT/DVE/Pool
                a1 = apool.tile([128, BL], F32, tag="a1", name="a1")
                nc.scalar.activation(a1[:], u[:],
                                     mybir.ActivationFunctionType.Relu,
                                     bias=negthr[:])
                m2 = clpool.tile([128, BL], F32, tag="clip", name="m2")
                nc.vector.tensor_scalar(m2[:], u[:], THR, 0.0, ADD, MINOP)
                nc.gpsimd.tensor_add(sh_tile.t[:], a1[:], m2[:])
                if s_mm_tile is not None:
                    # PE RNE-rounds raw fp32 bits on read (probe-verified),
                    # so a bit-copy into the f32r tile is equivalent to a
                    # rounding copy - and DMA engines are otherwise idle.
                    nc.sync.dma_start(s_mm_tile[:],
                                      sh_tile.t[:].bitcast(mmdt))

            class _Shadow:
                def __init__(self, t):
                    self.t = t
                    self.fresh = True

            shadow = [_Shadow(shpool.tile([128, BL], F32, tag="sh",
                                          name=f"sh{n}"))
                      for n in range(KCH)]

            # ---- iteration 1 (s0 = 0): u = A @ bT directly ----
            s_cur = [spool.tile([128, BL], mmdt, tag="sT", name=f"s0_{n}")
                     for n in range(KCH)]
            for n in range(KCH):
                ps2 = psB.tile([128, BL], F32, tag="psB", name="ps2")
                for m in range(MCH):
                    nc.tensor.matmul(
                        ps2[:],
                        at_t[:, m, n * 128:(n + 1) * 128],
                        bT[m][:],
                        start=(m == 0), stop=(m == MCH - 1))
                soft_update(ps2, shadow[n], s_cur[n])

            # ---- iterations 2..n_iters ----
            for it in range(1, n_iters):
                rT = [rpool.tile([128, BL], mmdt, tag="rT", name=f"rT{m}")
                      for m in range(MCH)]
                # k-major interleaved accumulation across 4 PSUM banks:
                # each s_mm chunk is consumed by 4 consecutive matmuls, so
                # the PE tracks the elementwise drain with slack.
                ps1s = [psA.tile([128, BL], F32, tag="psA", name=f"ps1_{m}")
                        for m in range(MCH)]
                for k in range(KCH):
                    for m in range(MCH):
                        nc.tensor.matmul(
                            ps1s[m][:],
                            a_t[:, k, m * 128:(m + 1) * 128],
                            s_cur[k][:],
                            start=(k == 0), stop=(k == KCH - 1))
                for m in range(MCH):
                    # rT' = bT - psum = (psum * -1) + bT, one DVE op
                    nc.vector.scalar_tensor_tensor(
                        rT[m][:], ps1s[m][:], -1.0, bT[m][:].bitcast(F32),
                        MULT, ADD)
                last = (it == n_iters - 1)
                for n in range(KCH):
                    ps2 = psB.tile([128, BL], F32, tag="psB", name="ps2")
                    for m in range(MCH):
                        nc.tensor.matmul(
                            ps2[:],
                            at_t[:, m, n * 128:(n + 1) * 128],
                            rT[m][:],
                            start=(m == 0), stop=(m == MCH - 1))
                    soft_update(ps2, shadow[n],
                                None if (last and fdt != mmdt) else s_cur[n])

            # ---- final: outT[n-block] = D[:,n-block]^T @ sT / SCALE ----
            with tc.tile_pool(name="dstr", bufs=4) as dpool:
                if fdt != mmdt:
                    s_cur = [sh.t for sh in shadow]
                for n in range(KCH):
                    d_t = dpool.tile([128, KCH, 128], fdt, tag="D", name="dstr")
                    nc.gpsimd.dma_start(d_t[:], d_d[n].rearrange(
                        "p (k c) -> p k c", k=KCH))
                    ps2 = psB.tile([128, BL], F32, tag="psB", name="ps2")
                    for k in range(KCH):
                        nc.tensor.matmul(
                            ps2[:],
                            d_t[:, k, :],
                            s_cur[k][:],
                            start=(k == 0), stop=(k == KCH - 1))
                    o = opool.tile([128, BL], F32, tag="o", name="o")
                    nc.vector.tensor_scalar(o[:], ps2[:], 1.0 / SCALE, None,
                                            MULT)
                    nc.sync.dma_start(o_d[n * 128:(n + 1) * 128, :], o[:])

    nc.compile()
    return nc


def _get_nc(n_iters=N_ITERS, use_f32r=True, final_f32r=True):
    key = (n_iters, use_f32r, final_f32r)
    if key not in _CACHE:
        _CACHE[key] = _build(*key)
    return _CACHE[key]


def _make_in_maps(x: np.ndarray, idxs: np.ndarray):
    idxs = np.asarray(idxs).astype(np.int64)
    D = _dct_matrix(N)
    A = D[:, idxs]                                   # [N, M]
    sel = np.zeros((N, M), dtype=np.float64)
    sel[idxs, np.arange(M)] = SCALE
    a_p = _pack(A.astype(np.float32), KCH)
    at_p = _pack(np.ascontiguousarray(A.T).astype(np.float32), MCH)
    self32 = sel.astype(np.float32)
    sel_p = np.stack([
        np.ascontiguousarray(
            self32[:, m * 128:(m + 1) * 128].reshape(KCH, 128, 128)
            .swapaxes(0, 1).reshape(128, KCH * 128))
        for m in range(MCH)])
    Df = D.astype(np.float32)
    d_p = np.stack([
        np.ascontiguousarray(
            Df[:, n * 128:(n + 1) * 128].reshape(KCH, 128, 128)
            .swapaxes(0, 1).reshape(128, KCH * 128))
        for n in range(KCH)])

    xf = np.asarray(x, dtype=np.float32).reshape(BC, N)
    in_maps = []
    for c in range(NCORES):
        shard = xf[c * BL:(c + 1) * BL, :]           # [BL, N]
        xt = np.ascontiguousarray(shard.T)           # [N, BL]
        in_maps.append({
            "xTpk": _pack(xt, KCH),
            "Apk": a_p,
            "ATpk": at_p,
            "SELpk": sel_p,
            "Dpk": d_p,
        })
    return in_maps


def _run(x, idxs, n_iters=N_ITERS, use_f32r=True, final_f32r=True,
         trace=False, **spmd_kwargs):
    nc = _get_nc(n_iters, use_f32r, final_f32r)
    in_maps = _make_in_maps(x, idxs)
    res = run_bass_kernel_spmd(nc, in_maps, list(range(NCORES)), trace=trace,
                               **spmd_kwargs)
    outs = []
    for c in range(NCORES):
        ot = res.results[c]["outT"]                  # [N, BL]
        outs.append(np.ascontiguousarray(ot.T))      # [BL, N]
    full = np.concatenate(outs, axis=0).reshape(B, CH, N).astype(np.float32)
    return full, res


def kernel(x, idxs):
    full, _ = _run(x, idxs)
    return (full,)


# revision 19
# speedup vs baseline: 1.0136x; 1.0024x over previous
"""TRN2 Bass kernel for batched compressed-sensing ISTA solver (nn_CS).

Reference semantics (per batch*channel signal of length N=2048, M=512
measurements at sorted unique indices `idxs`):
    b = SCALE * x[idxs]
    s_0 = 0
    repeat N_ITERS:                        # A = D[:, idxs], D = ortho DCT-II matrix
        r   = s @ A - b                    # A s  = idct(s)[idxs]
        s   = soft_threshold(s - r @ A.T, STEP*C_L1)
    out = (s @ D) / SCALE                  # idct(s) / SCALE

All 3072 solves are independent -> shard batch*channel over 8 NeuronCores
(384 rows each). Per core everything lives in SBUF; each iteration is two
matmul groups on the TensorEngine against the constant A (2048x512):
    p1[m]  = A[:,m-block]^T @ sT          (64 matmuls,  contraction N=2048)
    rT'    = bT - p1                      ( = -r^T )
    p2[n]  = A[n-block,:] @ rT'           (64 matmuls,  contraction M=512)
    u      = sT + p2                      ( = (s - r A^T)^T )
    sT     = u - clip(u, -t, t)           ( = soft_threshold(u, t) )
Matmuls run in float32r (full PE rate; fp32 runs at 1/4 rate) by default.

Everything is stored feature-major ([feature, batch] = partition x free);
host transposes x / output once (pure layout prep).
"""

import sys
import numpy as np

for _p in ("/opt/trn_rl_repo", "/root/.axon_site/_ro/trn_rl_repo"):
    if _p not in sys.path:
        sys.path.insert(0, _p)

import concourse.bass as bass  # noqa: E402
import concourse.bacc as bacc  # noqa: E402
import concourse.mybir as mybir  # noqa: E402
import concourse.tile as tile  # noqa: E402
from concourse.bass_utils import run_bass_kernel_spmd  # noqa: E402

# ---- problem constants (hardcoded per spec) --------------------------------
B, CH, N, M = 256, 12, 2048, 512
NCORES = 8
BC = B * CH                  # 3072 total solves
BL = BC // NCORES            # 384 solves per core
N_ITERS = 100
SCALE = 100.0
C_L1 = 0.1
STEP = 0.5
THR = STEP * C_L1            # 0.05 soft threshold
KCH = N // 128               # 16 chunks of the N axis
MCH = M // 128               # 4 chunks of the M axis

F32 = mybir.dt.float32
F32R = mybir.dt.float32r
ADD = mybir.AluOpType.add
MAXOP = mybir.AluOpType.max
MINOP = mybir.AluOpType.min
MULT = mybir.AluOpType.mult

_CACHE: dict = {}


def _dct_matrix(n: int) -> np.ndarray:
    """D with dct(v, norm='ortho') = D @ v; idct(v) = D.T @ v (row: s @ D)."""
    k = np.arange(n, dtype=np.float64)[:, None]
    j = np.arange(n, dtype=np.float64)[None, :]
    D = np.cos(np.pi * (2.0 * j + 1.0) * k / (2.0 * n))
    D[0, :] *= np.sqrt(1.0 / n)
    D[1:, :] *= np.sqrt(2.0 / n)
    return D


def _pack(mat: np.ndarray, nch: int) -> np.ndarray:
    """[nch*128, C] row-major -> [128, nch, C] partition-major SBUF layout."""
    r, c = mat.shape
    assert r == nch * 128
    return np.ascontiguousarray(
        mat.reshape(nch, 128, c).swapaxes(0, 1), dtype=np.float32
    )


def _build(n_iters: int, use_f32r: bool, final_f32r: bool):
    """Build + compile the per-core Bass program (identical on all cores)."""
    mmdt = F32R if use_f32r else F32
    fdt = F32R if final_f32r else F32

    nc = bacc.Bacc("TRN2", target_bir_lowering=False, debug=False,
                   num_devices=NCORES)

    x_d = nc.dram_tensor("xTpk", [128, KCH, BL], mmdt, kind="ExternalInput")
    a_d = nc.dram_tensor("Apk", [128, KCH, M], mmdt, kind="ExternalInput")
    at_d = nc.dram_tensor("ATpk", [128, MCH, N], mmdt, kind="ExternalInput")
    sel_d = nc.dram_tensor("SELpk", [MCH, 128, KCH * 128], mmdt,
                           kind="ExternalInput")
    d_d = nc.dram_tensor("Dpk", [KCH, 128, KCH * 128], fdt,
                         kind="ExternalInput")
    o_d = nc.dram_tensor("outT", [N, BL], F32, kind="ExternalOutput")

    with tile.TileContext(nc) as tc:
        with (
            tc.tile_pool(name="const", bufs=1) as cpool,
            tc.tile_pool(name="bT", bufs=MCH) as bpool,
            tc.tile_pool(name="sT", bufs=KCH) as spool,
            tc.tile_pool(name="sh", bufs=KCH) as shpool,
            tc.tile_pool(name="rT", bufs=2 * MCH) as rpool,
            tc.tile_pool(name="u", bufs=4) as upool,
            tc.tile_pool(name="clip", bufs=4) as clpool,
            tc.tile_pool(name="a1", bufs=4) as apool,
            tc.tile_pool(name="o", bufs=2) as opool,
            tc.tile_pool(name="psA", bufs=MCH, space="PSUM") as psA,
            tc.tile_pool(name="psB", bufs=4, space="PSUM") as psB,
        ):
            a_t = cpool.tile([128, KCH, M], mmdt, tag="A")
            at_t = cpool.tile([128, MCH, N], mmdt, tag="AT")

            negthr = cpool.tile([128, 1], F32, tag="negthr", name="negthr")
            nc.gpsimd.memset(negthr[:], -THR)

            bT = [bpool.tile([128, BL], mmdt, tag="bT", name=f"bT{m}")
                  for m in range(MCH)]

            # ---- init: bT[m] = (SCALE*Sel)^T @ xT (f32r; PE rounds) ----
            with (
                tc.tile_pool(name="initx", bufs=1) as xpool,
                tc.tile_pool(name="inits", bufs=2) as ipool,
            ):
                xfull = xpool.tile([128, KCH, BL], mmdt, tag="xk",
                                   name="xfull")
                nc.sync.dma_start(xfull[:], x_d[:])
                for m in range(MCH):
                    selm = ipool.tile([128, KCH * 128], mmdt, tag="selm",
                                      name=f"selm{m}")
                    nc.sync.dma_start(selm[:], sel_d[m])
                    ps = psA.tile([128, BL], F32, tag="psA", name="psA_b")
                    for k in range(KCH):
                        nc.tensor.matmul(ps[:],
                                         selm[:, k * 128:(k + 1) * 128],
                                         xfull[:, k, :],
                                         start=(k == 0), stop=(k == KCH - 1))
                    nc.vector.tensor_copy(bT[m][:], ps[:])
                # constant uploads, batched with per-slice deps
                for m in range(MCH):
                    nc.gpsimd.dma_start(at_t[:, m, :], at_d[:, m, :])
                for g in range(4):
                    nc.sync.dma_start(a_t[:, 4 * g:4 * g + 4, :],
                                      a_d[:, 4 * g:4 * g + 4, :])

            def soft_update(ps2, sh_tile, s_mm_tile):
                # shadow = soft_threshold(shadow + ps2, THR)  [fp32, exact]
                # s_mm   = round_f32r(shadow)                 [PE operand]
                u = upool.tile([128, BL], F32, tag="u", name="u")
                if sh_tile.fresh:
                    nc.vector.tensor_copy(u[:], ps2[:])
                    sh_tile.fresh = False
                else:
                    nc.vector.tensor_add(u[:], sh_tile.t[:], ps2[:])
                # soft(u) = relu(u-t) + min(u+t, 0), split across ACT/DVE/Pool
                a1 = apool.tile([128, BL], F32, tag="a1", name="a1")
                nc.scalar.activation(a1[:], u[:],
                                     mybir.ActivationFunctionType.Relu,
                                     bias=negthr[:])
                m2 = clpool.tile([128, BL], F32, tag="clip", name="m2")
                nc.vector.tensor_scalar(m2[:], u[:], THR, 0.0, ADD, MINOP)
                nc.gpsimd.tensor_add(sh_tile.t[:], a1[:], m2[:])
                if s_mm_tile is not None:
                    # PE RNE-rounds raw fp32 bits on read (probe-verified),
                    # so a bit-copy into the f32r tile is equivalent to a
                    # rounding copy - and DMA engines are otherwise idle.
                    nc.sync.dma_start(s_mm_tile[:],
                                      sh_tile.t[:].bitcast(mmdt))

            class _Shadow:
                def __init__(self, t):
                    self.t = t
                    self.fresh = True

            shadow = [_Shadow(shpool.tile([128, BL], F32, tag="sh",
                                          name=f"sh{n}"))
                      for n in range(KCH)]

            # ---- iteration 1 (s0 = 0): u = A @ bT directly ----
            s_cur = [spool.tile([128, BL], mmdt, tag="sT", name=f"s0_{n}")
                     for n in range(KCH)]
            for n in range(KCH):
                ps2 = psB.tile([128, BL], F32, tag="psB", name="ps2")
                for m in range(MCH):
                    nc.tensor.matmul(
                        ps2[:],
                        at_t[:, m, n * 128:(n + 1) * 128],
                        bT[m][:],
                        start=(m == 0), stop=(m == MCH - 1))
                soft_update(ps2, shadow[n], s_cur[n])

            # ---- iterations 2..n_iters ----
            for it in range(1, n_iters):
                rT = [rpool.tile([128, BL], mmdt, tag="rT", name=f"rT{m}")
                      for m in range(MCH)]
                # k-major interleaved accumulation across 4 PSUM banks:
                # each s_mm chunk is consumed by 4 consecutive matmuls, so
                # the PE tracks the elementwise drain with slack.
                ps1s = [psA.tile([128, BL], F32, tag="psA", name=f"ps1_{m}")
                        for m in range(MCH)]
                for k in range(KCH):
                    for m in range(MCH):
                        nc.tensor.matmul(
                            ps1s[m][:],
                            a_t[:, k, m * 128:(m + 1) * 128],
                            s_cur[k][:],
                            start=(k == 0), stop=(k == KCH - 1))
                for m in range(MCH):
                    # rT' = bT - psum = (psum * -1) + bT, one DVE op
                    nc.vector.scalar_tensor_tensor(
                        rT[m][:], ps1s[m][:], -1.0, bT[m][:].bitcast(F32),
                        MULT, ADD)
                last = (it == n_iters - 1)
                for n in range(KCH):
                    ps2 = psB.tile([128, BL], F32, tag="psB", name="ps2")
                    for m in range(MCH):
                        nc.tensor.matmul(
                            ps2[:],
                            at_t[:, m, n * 128:(n + 1) * 128],
                            rT[m][:],
                            start=(m == 0), stop=(m == MCH - 1))
                    soft_update(ps2, shadow[n],
                                None if (last and fdt != mmdt) else s_cur[n])

            # ---- final: outT[n-block] = D[:,n-block]^T @ sT / SCALE ----
            with tc.tile_pool(name="dstr", bufs=4) as dpool:
                if fdt != mmdt:
                    s_cur = [sh.t for sh in shadow]
                for n in range(KCH):
                    d_t = dpool.tile([128, KCH, 128], fdt, tag="D", name="dstr")
                    eng = nc.gpsimd if n % 2 == 0 else nc.sync
                    eng.dma_start(d_t[:], d_d[n].rearrange(
                        "p (k c) -> p k c", k=KCH))
                    ps2 = psB.tile([128, BL], F32, tag="psB", name="ps2")
                    for k in range(KCH):
                        nc.tensor.matmul(
                            ps2[:],
                            d_t[:, k, :],
                            s_cur[k][:],
                            start=(k == 0), stop=(k == KCH - 1))
                    o = opool.tile([128, BL], F32, tag="o", name="o")
                    nc.vector.tensor_scalar(o[:], ps2[:], 1.0 / SCALE, None,
                                            MULT)
                    nc.sync.dma_start(o_d[n * 128:(n + 1) * 128, :], o[:])

    nc.compile()
    return nc


def _get_nc(n_iters=N_ITERS, use_f32r=True, final_f32r=True):
    key = (n_iters, use_f32r, final_f32r)
    if key not in _CACHE:
        _CACHE[key] = _build(*key)
    return _CACHE[key]


def _make_in_maps(x: np.ndarray, idxs: np.ndarray):
    idxs = np.asarray(idxs).astype(np.int64)
    D = _dct_matrix(N)
    A = D[:, idxs]                                   # [N, M]
    sel = np.zeros((N, M), dtype=np.float64)
    sel[idxs, np.arange(M)] = SCALE
    a_p = _pack(A.astype(np.float32), KCH)
    at_p = _pack(np.ascontiguousarray(A.T).astype(np.float32), MCH)
    self32 = sel.astype(np.float32)
    sel_p = np.stack([
        np.ascontiguousarray(
            self32[:, m * 128:(m + 1) * 128].reshape(KCH, 128, 128)
            .swapaxes(0, 1).reshape(128, KCH * 128))
        for m in range(MCH)])
    Df = D.astype(np.float32)
    d_p = np.stack([
        np.ascontiguousarray(
            Df[:, n * 128:(n + 1) * 128].reshape(KCH, 128, 128)
            .swapaxes(0, 1).reshape(128, KCH * 128))
        for n in range(KCH)])

    xf = np.asarray(x, dtype=np.float32).reshape(BC, N)
    in_maps = []
    for c in range(NCORES):
        shard = xf[c * BL:(c + 1) * BL, :]           # [BL, N]
        xt = np.ascontiguousarray(shard.T)           # [N, BL]
        in_maps.append({
            "xTpk": _pack(xt, KCH),
            "Apk": a_p,
            "ATpk": at_p,
            "SELpk": sel_p,
            "Dpk": d_p,
        })
    return in_maps


def _run(x, idxs, n_iters=N_ITERS, use_f32r=True, final_f32r=True,
         trace=False, **spmd_kwargs):
    nc = _get_nc(n_iters, use_f32r, final_f32r)
    in_maps = _make_in_maps(x, idxs)
    res = run_bass_kernel_spmd(nc, in_maps, list(range(NCORES)), trace=trace,
                               **spmd_kwargs)
    outs = []
    for c in range(NCORES):
        ot = res.results[c]["outT"]                  # [N, BL]
        outs.append(np.ascontiguousarray(ot.T))      # [BL, N]
    full = np.concatenate(outs, axis=0).reshape(B, CH, N).astype(np.float32)
    return full, res


def kernel(x, idxs):
    full, _ = _run(x, idxs)
    return (full,)


# revision 21
# speedup vs baseline: 1.0423x; 1.0283x over previous
"""TRN2 Bass kernel for batched compressed-sensing ISTA solver (nn_CS).

Reference semantics (per batch*channel signal of length N=2048, M=512
measurements at sorted unique indices `idxs`):
    b = SCALE * x[idxs]
    s_0 = 0
    repeat N_ITERS:                        # A = D[:, idxs], D = ortho DCT-II matrix
        r   = s @ A - b                    # A s  = idct(s)[idxs]
        s   = soft_threshold(s - r @ A.T, STEP*C_L1)
    out = (s @ D) / SCALE                  # idct(s) / SCALE

All 3072 solves are independent -> shard batch*channel over 8 NeuronCores
(384 rows each). Per core everything lives in SBUF; each iteration is two
matmul groups on the TensorEngine against the constant A (2048x512):
    p1[m]  = A[:,m-block]^T @ sT          (64 matmuls,  contraction N=2048)
    rT'    = bT - p1                      ( = -r^T )
    p2[n]  = A[n-block,:] @ rT'           (64 matmuls,  contraction M=512)
    u      = sT + p2                      ( = (s - r A^T)^T )
    sT     = u - clip(u, -t, t)           ( = soft_threshold(u, t) )
Matmuls run in float32r (full PE rate; fp32 runs at 1/4 rate) by default.

Everything is stored feature-major ([feature, batch] = partition x free);
host transposes x / output once (pure layout prep).
"""

import sys
import numpy as np

for _p in ("/opt/trn_rl_repo", "/root/.axon_site/_ro/trn_rl_repo"):
    if _p not in sys.path:
        sys.path.insert(0, _p)

import concourse.bass as bass  # noqa: E402
import concourse.bacc as bacc  # noqa: E402
import concourse.mybir as mybir  # noqa: E402
import concourse.tile as tile  # noqa: E402
from concourse.bass_utils import run_bass_kernel_spmd  # noqa: E402

# ---- problem constants (hardcoded per spec) --------------------------------
B, CH, N, M = 256, 12, 2048, 512
NCORES = 8
BC = B * CH                  # 3072 total solves
BL = BC // NCORES            # 384 solves per core
N_ITERS = 100
SCALE = 100.0
C_L1 = 0.1
STEP = 0.5
THR = STEP * C_L1            # 0.05 soft threshold
KCH = N // 128               # 16 chunks of the N axis
MCH = M // 128               # 4 chunks of the M axis

F32 = mybir.dt.float32
F32R = mybir.dt.float32r
ADD = mybir.AluOpType.add
MAXOP = mybir.AluOpType.max
MINOP = mybir.AluOpType.min
MULT = mybir.AluOpType.mult

_CACHE: dict = {}


def _dct_matrix(n: int) -> np.ndarray:
    """D with dct(v, norm='ortho') = D @ v; idct(v) = D.T @ v (row: s @ D)."""
    k = np.arange(n, dtype=np.float64)[:, None]
    j = np.arange(n, dtype=np.float64)[None, :]
    D = np.cos(np.pi * (2.0 * j + 1.0) * k / (2.0 * n))
    D[0, :] *= np.sqrt(1.0 / n)
    D[1:, :] *= np.sqrt(2.0 / n)
    return D


def _pack(mat: np.ndarray, nch: int) -> np.ndarray:
    """[nch*128, C] row-major -> [128, nch, C] partition-major SBUF layout."""
    r, c = mat.shape
    assert r == nch * 128
    return np.ascontiguousarray(
        mat.reshape(nch, 128, c).swapaxes(0, 1), dtype=np.float32
    )


def _build(n_iters: int, use_f32r: bool, final_f32r: bool):
    """Build + compile the per-core Bass program (identical on all cores)."""
    mmdt = F32R if use_f32r else F32
    fdt = F32R if final_f32r else F32

    nc = bacc.Bacc("TRN2", target_bir_lowering=False, debug=False,
                   num_devices=NCORES)

    x_d = nc.dram_tensor("xTpk", [128, KCH, BL], mmdt, kind="ExternalInput")
    a_d = nc.dram_tensor("Apk", [128, KCH, M], mmdt, kind="ExternalInput")
    at_d = nc.dram_tensor("ATpk", [128, MCH, N], mmdt, kind="ExternalInput")
    sel_d = nc.dram_tensor("SELpk", [MCH, 128, KCH * 128], mmdt,
                           kind="ExternalInput")
    d_d = nc.dram_tensor("Dpk", [KCH, 128, KCH * 128], fdt,
                         kind="ExternalInput")
    o_d = nc.dram_tensor("outT", [N, BL], F32, kind="ExternalOutput")

    with tile.TileContext(nc) as tc:
        with (
            tc.tile_pool(name="const", bufs=1) as cpool,
            tc.tile_pool(name="bT", bufs=MCH) as bpool,
            tc.tile_pool(name="sT", bufs=KCH) as spool,
            tc.tile_pool(name="sh", bufs=KCH) as shpool,
            tc.tile_pool(name="rT", bufs=2 * MCH) as rpool,
            tc.tile_pool(name="u", bufs=5) as upool,
            tc.tile_pool(name="clip", bufs=5) as clpool,
            tc.tile_pool(name="a1", bufs=5) as apool,
            tc.tile_pool(name="o", bufs=2) as opool,
            tc.tile_pool(name="ps", bufs=8, space="PSUM") as pspool,
        ):
            a_t = cpool.tile([128, KCH, M], mmdt, tag="A")
            at_t = cpool.tile([128, MCH, N], mmdt, tag="AT")

            negthr = cpool.tile([128, 1], F32, tag="negthr", name="negthr")
            nc.gpsimd.memset(negthr[:], -THR)

            bT = [bpool.tile([128, BL], mmdt, tag="bT", name=f"bT{m}")
                  for m in range(MCH)]

            # ---- init: bT[m] = (SCALE*Sel)^T @ xT (f32r; PE rounds) ----
            with (
                tc.tile_pool(name="initx", bufs=1) as xpool,
                tc.tile_pool(name="inits", bufs=2) as ipool,
            ):
                xfull = xpool.tile([128, KCH, BL], mmdt, tag="xk",
                                   name="xfull")
                nc.sync.dma_start(xfull[:], x_d[:])
                for m in range(MCH):
                    selm = ipool.tile([128, KCH * 128], mmdt, tag="selm",
                                      name=f"selm{m}")
                    nc.sync.dma_start(selm[:], sel_d[m])
                    ps = pspool.tile([128, BL], F32, tag="ps", name="psA_b")
                    for k in range(KCH):
                        nc.tensor.matmul(ps[:],
                                         selm[:, k * 128:(k + 1) * 128],
                                         xfull[:, k, :],
                                         start=(k == 0), stop=(k == KCH - 1))
                    nc.vector.tensor_copy(bT[m][:], ps[:])
                # constant uploads, batched with per-slice deps
                for m in range(MCH):
                    nc.gpsimd.dma_start(at_t[:, m, :], at_d[:, m, :])
                for g in range(4):
                    nc.sync.dma_start(a_t[:, 4 * g:4 * g + 4, :],
                                      a_d[:, 4 * g:4 * g + 4, :])

            def soft_update(ps2, sh_tile, s_mm_tile):
                # shadow = soft_threshold(shadow + ps2, THR)  [fp32, exact]
                # s_mm   = round_f32r(shadow)                 [PE operand]
                u = upool.tile([128, BL], F32, tag="u", name="u")
                if sh_tile.fresh:
                    nc.vector.tensor_copy(u[:], ps2[:])
                    sh_tile.fresh = False
                else:
                    nc.vector.tensor_add(u[:], sh_tile.t[:], ps2[:])
                # soft(u) = relu(u-t) + min(u+t, 0), split across ACT/DVE/Pool
                a1 = apool.tile([128, BL], F32, tag="a1", name="a1")
                nc.scalar.activation(a1[:], u[:],
                                     mybir.ActivationFunctionType.Relu,
                                     bias=negthr[:])
                m2 = clpool.tile([128, BL], F32, tag="clip", name="m2")
                nc.vector.tensor_scalar(m2[:], u[:], THR, 0.0, ADD, MINOP)
                nc.gpsimd.tensor_add(sh_tile.t[:], a1[:], m2[:])
                if s_mm_tile is not None:
                    # PE RNE-rounds raw fp32 bits on read (probe-verified),
                    # so a bit-copy into the f32r tile is equivalent to a
                    # rounding copy - and DMA engines are otherwise idle.
                    nc.sync.dma_start(s_mm_tile[:],
                                      sh_tile.t[:].bitcast(mmdt))

            class _Shadow:
                def __init__(self, t):
                    self.t = t
                    self.fresh = True

            shadow = [_Shadow(shpool.tile([128, BL], F32, tag="sh",
                                          name=f"sh{n}"))
                      for n in range(KCH)]

            # ---- iteration 1 (s0 = 0): u = A @ bT directly ----
            s_cur = [spool.tile([128, BL], mmdt, tag="sT", name=f"s0_{n}")
                     for n in range(KCH)]
            for n in range(KCH):
                ps2 = pspool.tile([128, BL], F32, tag="ps", name="ps2")
                for m in range(MCH):
                    nc.tensor.matmul(
                        ps2[:],
                        at_t[:, m, n * 128:(n + 1) * 128],
                        bT[m][:],
                        start=(m == 0), stop=(m == MCH - 1))
                soft_update(ps2, shadow[n], s_cur[n])

            # ---- iterations 2..n_iters ----
            for it in range(1, n_iters):
                rT = [rpool.tile([128, BL], mmdt, tag="rT", name=f"rT{m}")
                      for m in range(MCH)]
                # k-major interleaved accumulation across 4 PSUM banks:
                # each s_mm chunk is consumed by 4 consecutive matmuls, so
                # the PE tracks the elementwise drain with slack.
                ps1s = [pspool.tile([128, BL], F32, tag="ps", name=f"ps1_{m}")
                        for m in range(MCH)]
                for k in range(KCH):
                    for m in range(MCH):
                        nc.tensor.matmul(
                            ps1s[m][:],
                            a_t[:, k, m * 128:(m + 1) * 128],
                            s_cur[k][:],
                            start=(k == 0), stop=(k == KCH - 1))
                for m in range(MCH):
                    # rT' = bT - psum = (psum * -1) + bT, one DVE op
                    nc.vector.scalar_tensor_tensor(
                        rT[m][:], ps1s[m][:], -1.0, bT[m][:].bitcast(F32),
                        MULT, ADD)
                last = (it == n_iters - 1)
                for n in range(KCH):
                    ps2 = pspool.tile([128, BL], F32, tag="ps", name="ps2")
                    for m in range(MCH):
                        nc.tensor.matmul(
                            ps2[:],
                            at_t[:, m, n * 128:(n + 1) * 128],
                            rT[m][:],
                            start=(m == 0), stop=(m == MCH - 1))
                    soft_update(ps2, shadow[n],
                                None if (last and fdt != mmdt) else s_cur[n])

            # ---- final: outT[n-block] = D[:,n-block]^T @ sT / SCALE ----
            with tc.tile_pool(name="dstr", bufs=4) as dpool:
                if fdt != mmdt:
                    s_cur = [sh.t for sh in shadow]
                for n in range(KCH):
                    d_t = dpool.tile([128, KCH, 128], fdt, tag="D", name="dstr")
                    eng = nc.gpsimd if n % 2 == 0 else nc.sync
                    eng.dma_start(d_t[:], d_d[n].rearrange(
                        "p (k c) -> p k c", k=KCH))
                    ps2 = pspool.tile([128, BL], F32, tag="ps", name="ps2")
                    for k in range(KCH):
                        nc.tensor.matmul(
                            ps2[:],
                            d_t[:, k, :],
                            s_cur[k][:],
                            start=(k == 0), stop=(k == KCH - 1))
                    o = opool.tile([128, BL], F32, tag="o", name="o")
                    nc.vector.tensor_scalar(o[:], ps2[:], 1.0 / SCALE, None,
                                            MULT)
                    nc.sync.dma_start(o_d[n * 128:(n + 1) * 128, :], o[:])

    nc.compile()
    return nc


def _get_nc(n_iters=N_ITERS, use_f32r=True, final_f32r=True):
    key = (n_iters, use_f32r, final_f32r)
    if key not in _CACHE:
        _CACHE[key] = _build(*key)
    return _CACHE[key]


def _make_in_maps(x: np.ndarray, idxs: np.ndarray):
    idxs = np.asarray(idxs).astype(np.int64)
    D = _dct_matrix(N)
    A = D[:, idxs]                                   # [N, M]
    sel = np.zeros((N, M), dtype=np.float64)
    sel[idxs, np.arange(M)] = SCALE
    a_p = _pack(A.astype(np.float32), KCH)
    at_p = _pack(np.ascontiguousarray(A.T).astype(np.float32), MCH)
    self32 = sel.astype(np.float32)
    sel_p = np.stack([
        np.ascontiguousarray(
            self32[:, m * 128:(m + 1) * 128].reshape(KCH, 128, 128)
            .swapaxes(0, 1).reshape(128, KCH * 128))
        for m in range(MCH)])
    Df = D.astype(np.float32)
    d_p = np.stack([
        np.ascontiguousarray(
            Df[:, n * 128:(n + 1) * 128].reshape(KCH, 128, 128)
            .swapaxes(0, 1).reshape(128, KCH * 128))
        for n in range(KCH)])

    xf = np.asarray(x, dtype=np.float32).reshape(BC, N)
    in_maps = []
    for c in range(NCORES):
        shard = xf[c * BL:(c + 1) * BL, :]           # [BL, N]
        xt = np.ascontiguousarray(shard.T)           # [N, BL]
        in_maps.append({
            "xTpk": _pack(xt, KCH),
            "Apk": a_p,
            "ATpk": at_p,
            "SELpk": sel_p,
            "Dpk": d_p,
        })
    return in_maps


def _run(x, idxs, n_iters=N_ITERS, use_f32r=True, final_f32r=True,
         trace=False, **spmd_kwargs):
    nc = _get_nc(n_iters, use_f32r, final_f32r)
    in_maps = _make_in_maps(x, idxs)
    res = run_bass_kernel_spmd(nc, in_maps, list(range(NCORES)), trace=trace,
                               **spmd_kwargs)
    outs = []
    for c in range(NCORES):
        ot = res.results[c]["outT"]                  # [N, BL]
        outs.append(np.ascontiguousarray(ot.T))      # [BL, N]
    full = np.concatenate(outs, axis=0).reshape(B, CH, N).astype(np.float32)
    return full, res


def kernel(x, idxs):
    full, _ = _run(x, idxs)
    return (full,)
